# revision 1
# baseline (speedup 1.0000x reference)
"""Matryoshka soft-top-k gating kernel for Trainium2 (Bass/Tile).

Computes, for each matryoshka scale k in (128, 64, 32):
    scores  = emb @ w  (+ b, which cancels in scores - threshold and is
              skipped)
    scores  = where(mask, scores, -BIG)
    thr_k   = k-th largest score per row
    diff    = clip(scores - thr_k, -50, 50)
    gate    = sigmoid(diff * temperature) * mask
    out_k   = emb * gate[..., None]

Sharding: data-parallel over the batch axis across 8 NeuronCores
(64 rows per core); w/temperature replicated, mask sharded with batch.

Per-core structure: embeddings stay SBUF-resident (8 MB). Scores are
computed token-major with a fused multiply+reduce (scalar_tensor_tensor
with accum_out), split between DVE and Pool so they track the input DMA.
The 64 rows are processed as two row groups of 32 pipelined against each
other: group A's thresholds (max8 + match_replace top-8 extraction
chain) and its k=32 stores begin while group B's chunks are still
arriving, so the store pipe opens ~20 us before all input lands and the
DMA engines never sit idle between the load and store phases. Gating is
spread over ACT (k=32), DVE (k=64) and Pool (k=128), and the 48 stores
are issued in estimated-ready order so no stream blocks another on the
single SP sequencer.
"""

import numpy as np

import concourse.bacc as bacc
import concourse.bass as bass
import concourse.mybir as mybir
import concourse.tile as tile
from concourse.bass_utils import run_bass_kernel_spmd

N_CORES = 8
B, T, D = 512, 256, 128
R = B // N_CORES          # rows (documents) per core
KS = (128, 64, 32)
CLAMP = 50.0
BIG = 3.4e38              # stands in for -inf in masked_fill
REPL = -3.0e38            # match_replace sentinel (> -BIG)
NT = R * T // 128         # 128-token tiles per core
CH = 8                    # tiles per DMA chunk (512 KB)
NCH = NT // CH
# two pipelined row groups (asymmetric: A small so its stores open the
# store pipe while B is still streaming in)
GROUP_CHUNKS = ((0, 6), (6, 16))      # chunk [lo, hi) per group
NG = len(GROUP_CHUNKS)
ROWS_PER_CHUNK = CH // 2              # 4 rows per 8-tile chunk

f32 = mybir.dt.float32
Alu = mybir.AluOpType
Act = mybir.ActivationFunctionType



# store-issue order: estimated production time (us) per (k, chunk),
# calibrated against the TimelineSim schedule
STORE_EST = {
    (32, 0): 28.7,
    (32, 1): 29.7,
    (32, 2): 30.0,
    (32, 3): 34.1,
    (32, 4): 37.0,
    (32, 5): 40.1,
    (32, 6): 57.3,
    (32, 7): 59.6,
    (32, 8): 61.9,
    (32, 9): 64.3,
    (32, 10): 67.4,
    (32, 11): 69.7,
    (32, 12): 72.1,
    (32, 13): 74.4,
    (32, 14): 72.6,
    (32, 15): 74.1,
    (64, 0): 48.5,
    (64, 1): 50.8,
    (64, 2): 51.7,
    (64, 3): 57.7,
    (64, 4): 61.4,
    (64, 5): 64.5,
    (64, 6): 67.4,
    (64, 7): 68.4,
    (64, 8): 69.4,
    (64, 9): 70.4,
    (64, 10): 71.5,
    (64, 11): 75.5,
    (64, 12): 78.6,
    (64, 13): 80.0,
    (64, 14): 82.9,
    (64, 15): 85.8,
    (128, 0): 49.0,
    (128, 1): 51.2,
    (128, 2): 53.4,
    (128, 3): 55.6,
    (128, 4): 57.8,
    (128, 5): 59.9,
    (128, 6): 68.8,
    (128, 7): 71.0,
    (128, 8): 73.1,
    (128, 9): 75.3,
    (128, 10): 77.5,
    (128, 11): 79.7,
    (128, 12): 82.6,
    (128, 13): 88.4,
    (128, 14): 93.1,
    (128, 15): 97.5,
}


LAST_GATE_INST = {}


def _bcast(ap, n):
    """Replicate a DRAM vector across n partitions (0-step leading dim)."""
    return bass.AP(tensor=ap.tensor, offset=ap.offset, ap=[[0, n]] + list(ap.ap))


def build_bass(dma_only=False):
    """dma_only builds a store-input-3x timing variant (wrong numerics)."""
    nc = bacc.Bacc("TRN2", target_bir_lowering=False, debug=False)

    emb = nc.dram_tensor("embeddings", [R, T, D], f32, kind="ExternalInput")
    w = nc.dram_tensor("w", [D], f32, kind="ExternalInput")
    temp = nc.dram_tensor("temperature", [1], f32, kind="ExternalInput")
    mask = nc.dram_tensor("mask", [R, T], mybir.dt.uint8, kind="ExternalInput")
    out = nc.dram_tensor("out", [len(KS), R, T, D], f32, kind="ExternalOutput")
    ident_d = nc.inline_tensor(np.eye(128, dtype=np.float32), name="ident128")

    # token-major views: partition = flat_token % 128, free = (tile, d)
    emb_t = emb.ap().rearrange("r t d -> (r t) d").rearrange(
        "(n p) d -> p n d", p=128
    )
    out_t = out.ap().rearrange("k r t d -> k (r t) d").rearrange(
        "k (n p) d -> k p n d", p=128
    )

    with tile.TileContext(nc) as tc:
        with (
            tc.tile_pool(name="singles", bufs=1) as singles,
            tc.tile_pool(name="echunks", bufs=NCH) as epool,
            tc.tile_pool(name="out0", bufs=6) as opool0,
            tc.tile_pool(name="out1", bufs=6) as opool1,
            tc.tile_pool(name="out2", bufs=6) as opool2,
            tc.tile_pool(name="psum", bufs=4, space="PSUM") as psum,
        ):
            opools = [opool0, opool1, opool2]
            st = _State(nc, singles, psum, opools, out_t, ident=None)

            # ---- constants (HWDGE so they land within ~1 us) ----
            w_stage = singles.tile([128, D], f32)
            nc.sync.dma_start(out=w_stage, in_=_bcast(w.ap(), 128))
            w_rep = singles.tile([128, D], f32)
            nc.vector.tensor_copy(w_rep, w_stage)
            ident = singles.tile([128, 128], f32)
            nc.sync.dma_start(out=ident, in_=ident_d.ap())
            st.ident = ident
            # w along partitions, for the PE matvec score path
            w_col = singles.tile([128, 1], f32)
            nc.sync.dma_start(
                out=w_col,
                in_=bass.AP(tensor=w.ap().tensor, offset=0,
                            ap=[[1, 128], [0, 1]]),
            )
            # per-group mask/fill/temperature tiles (SBUF accesses must
            # start at partition 0, so shared [R, T] tiles cannot be
            # partition-sliced at group offsets)
            for g in range(NG):
                rg = _grows(g)
                tc_g = singles.tile([rg, 1], f32, tag=f"tc{g}", name=f"tc{g}")
                nc.sync.dma_start(out=tc_g, in_=_bcast(temp.ap(), rg))
                st.temp_col[g] = tc_g
                mu = singles.tile([rg, T], mybir.dt.uint8, tag=f"mu{g}",
                                  name=f"mu{g}")
                nc.sync.dma_start(out=mu, in_=mask.ap()[_rows(g), :])
                mf = singles.tile([rg, T], f32, tag=f"mf{g}", name=f"mf{g}")
                nc.vector.tensor_copy(mf, mu)
                st.mask_f[g] = mf
                ft = singles.tile([rg, T], f32, tag=f"ft{g}", name=f"ft{g}")
                nc.vector.tensor_scalar(
                    out=ft, in0=mf, scalar1=1.0, scalar2=BIG,
                    op0=Alu.subtract, op1=Alu.mult,
                )
                st.fillt[g] = ft
            # pre-load the ScalarE Sigmoid table off the critical path
            sig_warm = singles.tile([1, 1], f32)
            nc.scalar.activation(sig_warm, st.temp_col[0][0:1, :],
                                 Act.Sigmoid, bias=0.0, scale=1.0)

            # per-group score tiles: [128 tokens, RG] (col = group-local row)
            for g in range(NG):
                rg = _grows(g)
                lo, hi = GROUP_CHUNKS[g]
                st.scores_tm[g] = singles.tile([128, (hi - lo) * CH], f32,
                                               tag=f"stm{g}", name=f"stm{g}")
                st.scores_e[g] = singles.tile([128, rg], f32, tag=f"sce{g}", name=f"sce{g}")
                st.scores_o[g] = singles.tile([128, rg], f32, tag=f"sco{g}", name=f"sco{g}")
            trash_v = singles.tile([128, 1], f32)

            def load_and_score(g, dve_tiles, pe_tiles=3):
                """DMA group g's chunks; scores via three engine paths:
                DVE fused mul+reduce (STT accum), PE transpose+matvec, and
                Pool TT with a chunked DVE reduce for the remainder."""
                lo, hi = GROUP_CHUNKS[g]
                stm = st.scores_tm[g]
                for ci in range(lo, hi):
                    ech = epool.tile([128, CH, D], f32, tag="ech")
                    nc.sync.dma_start(
                        out=ech, in_=emb_t[:, ci * CH:(ci + 1) * CH, :]
                    )
                    st.echunks[ci] = ech
                    if dma_only:
                        continue
                    lc0 = (ci - lo) * CH
                    # DVE: fused multiply+reduce per tile
                    for j in range(dve_tiles):
                        nc.vector.scalar_tensor_tensor(
                            out=trash_v.broadcast_to([128, D]),
                            in0=ech[:, j, :],
                            scalar=1.0,
                            in1=w_rep,
                            op0=Alu.mult,
                            op1=Alu.mult,
                            accum_out=stm[:, lc0 + j:lc0 + j + 1],
                        )
                    # PE: transpose tile then matvec against w_col
                    npe = min(pe_tiles, CH - dve_tiles)
                    if npe:
                        psc = psum.tile([128, 8], f32, tag="psc", name="psc",
                                        bufs=2)
                        for i in range(npe):
                            j = dve_tiles + i
                            pet = psum.tile([128, 128], f32, tag="pet",
                                            name="pet", bufs=2)
                            nc.tensor.transpose(pet, ech[:, j, :], ident)
                            ebT = epool.tile([128, 128], f32, tag="ebT",
                                             name="ebT", bufs=3)
                            nc.scalar.copy(ebT, pet)
                            nc.tensor.matmul(psc[:, i:i + 1], ebT, w_col)
                        nc.vector.tensor_copy(
                            stm[:, lc0 + dve_tiles:lc0 + dve_tiles + npe],
                            psc[:, 0:npe],
                        )
                    # Pool: plain TT multiply, then one chunked DVE reduce
                    npool = CH - dve_tiles - npe
                    if npool:
                        j0 = dve_tiles + npe
                        prod = epool.tile([128, npool, D], f32, tag="prod",
                                          name="prod", bufs=2)
                        w_b = bass.AP(
                            tensor=w_rep.tensor, offset=w_rep.offset,
                            ap=[list(w_rep.ap[0]), [0, npool],
                                list(w_rep.ap[1])],
                        )
                        nc.gpsimd.tensor_tensor(
                            out=prod, in0=ech[:, j0:j0 + npool, :], in1=w_b,
                            op=Alu.mult,
                        )
                        nc.vector.tensor_reduce(
                            stm[:, lc0 + j0:lc0 + j0 + npool], prod,
                            axis=mybir.AxisListType.X, op=Alu.add,
                        )

            def deinterleave_scores(g):
                stm = st.scores_tm[g]
                rg = _grows(g)
                se = bass.AP(tensor=stm.tensor, offset=stm.offset,
                             ap=[list(stm.ap[0]), [2, rg]])
                so = bass.AP(tensor=stm.tensor, offset=stm.offset + 1,
                             ap=[list(stm.ap[0]), [2, rg]])
                nc.vector.tensor_copy(st.scores_e[g], se)
                nc.vector.tensor_copy(st.scores_o[g], so)

            st.deinterleave = deinterleave_scores
            if dma_only:
                for g in range(NG):
                    load_and_score(g, 0, 0)
                for k_i in range(len(KS)):
                    for ci in range(NCH):
                        nc.sync.dma_start(
                            out=out_t[k_i, :, ci * CH:(ci + 1) * CH, :],
                            in_=st.echunks[ci],
                        )
            else:
                _emit_pipeline(st, load_and_score)

    nc.compile()
    return nc


class _State:
    def __init__(self, nc, singles, psum, opools, out_t, ident):
        self.nc = nc
        self.singles = singles
        self.psum = psum
        self.opools = opools
        self.out_t = out_t
        self.ident = ident
        self.temp_col = {}
        self.mask_f = {}
        self.fillt = {}
        self.scores_tm = {}
        self.scores_e = {}
        self.scores_o = {}
        self.echunks = {}
        self.scores_msk = {}
        self.work = {}
        self.rounds_done = {}
        self.last_mx = {}
        self.masks_tm = {}
        self.ochs = {}


def _glo(g):
    return GROUP_CHUNKS[g][0] * ROWS_PER_CHUNK


def _grows(g):
    lo, hi = GROUP_CHUNKS[g]
    return (hi - lo) * ROWS_PER_CHUNK


def _rows(g):
    return slice(_glo(g), _glo(g) + _grows(g))


def _prefix(st, g):
    """Transpose group scores to row-major and apply masked_fill."""
    nc = st.nc
    rg = _grows(g)
    scores_rm = st.singles.tile([rg, T], f32, tag=f"srm{g}", name=f"srm{g}")
    pse = st.psum.tile([rg, 128], f32, tag="pse", name="pse", bufs=2)
    nc.tensor.transpose(pse, st.scores_e[g], st.ident)
    nc.vector.tensor_copy(scores_rm[:, 0:128], pse)
    pso = st.psum.tile([rg, 128], f32, tag="pse", name="pso", bufs=2)
    nc.tensor.transpose(pso, st.scores_o[g], st.ident)
    nc.vector.tensor_copy(scores_rm[:, 128:256], pso)

    prodt = st.singles.tile([rg, T], f32, tag=f"prod{g}", name=f"prod{g}")
    nc.vector.tensor_mul(prodt, scores_rm, st.mask_f[g])
    scores_msk = st.singles.tile([rg, T], f32, tag=f"smsk{g}", name=f"smsk{g}")
    nc.vector.tensor_add(scores_msk, prodt, st.fillt[g])
    st.scores_msk[g] = scores_msk
    st.work[g] = st.singles.tile([rg, T], f32, tag=f"work{g}", name=f"work{g}")
    st.rounds_done[g] = 0


def _rounds(st, g, upto):
    """Advance group g's top-8 extraction chain to `upto` rounds."""
    nc = st.nc
    n_rounds = max(KS) // 8
    while st.rounds_done[g] < upto:
        r = st.rounds_done[g]
        mx = st.singles.tile([_grows(g), 8], f32, tag=f"mx{g}_{r}", name=f"mx{g}_{r}")
        src = st.scores_msk[g] if r == 0 else st.work[g]
        nc.vector.max(out=mx, in_=src)
        if r < n_rounds - 1:
            nc.vector.match_replace(
                out=st.work[g], in_to_replace=mx, in_values=src,
                imm_value=REPL,
            )
        st.last_mx[g] = mx
        st.rounds_done[g] += 1
    return st.last_mx[g][:, 7:8]


def _dif(st, g, k, thr):
    """diff+clip for (g, k) on DVE (clip cannot bind for finite scores;
    the min keeps masked +BIG diffs from overflowing the sigmoid)."""
    nc = st.nc
    dif = st.singles.tile([_grows(g), T], f32, tag=f"dif{g}_{k}", name=f"dif{g}_{k}")
    nc.vector.tensor_scalar(
        out=dif, in0=st.scores_msk[g], scalar1=thr, scalar2=CLAMP,
        op0=Alu.subtract, op1=Alu.min,
    )
    return dif


def _sig_and_masks(st, g, k, dif):
    """sigmoid(temp*dif)*mask on ACT, then transpose to token-major cols."""
    nc = st.nc
    nc.scalar.activation(dif, dif, Act.Sigmoid, bias=0.0,
                         scale=st.temp_col[g])
    nc.vector.tensor_mul(dif, dif, st.mask_f[g])
    rg = _grows(g)
    me = st.singles.tile([128, rg], f32, tag=f"me{g}_{k}", name=f"me{g}_{k}")
    mo = st.singles.tile([128, rg], f32, tag=f"mo{g}_{k}", name=f"mo{g}_{k}")
    pme = st.psum.tile([128, rg], f32, tag="psm", name="pme", bufs=2)
    nc.tensor.transpose(pme, dif[:, 0:128], st.ident[:rg, :rg])
    nc.vector.tensor_copy(me, pme)
    pmo = st.psum.tile([128, rg], f32, tag="psm", name="pmo", bufs=2)
    nc.tensor.transpose(pmo, dif[:, 128:256], st.ident[:rg, :rg])
    nc.vector.tensor_copy(mo, pmo)
    st.masks_tm[(g, k)] = (me, mo)


def _gate(st, k, cis, engine_name):
    """Gate chunks `cis` for scale k on the named engine."""
    nc = st.nc
    k_i = KS.index(k)
    for ci in cis:
        g = next(i for i, (lo, hi) in enumerate(GROUP_CHUNKS) if lo <= ci < hi)
        me, mo = st.masks_tm[(g, k)]
        ech = st.echunks[ci]
        och = st.opools[k_i].tile([128, CH, D], f32, tag=f"och{k_i}")
        st.ochs[(k, ci)] = och
        for j in range(CH):
            c = ci * CH + j
            m = c // 2 - _glo(g)
            col = (me if c % 2 == 0 else mo)[:, m:m + 1]
            if engine_name == "scalar":
                ins = nc.scalar.activation(
                    och[:, j, :], ech[:, j, :], Act.Copy, bias=0.0, scale=col,
                )
            else:
                ins = getattr(nc, engine_name).tensor_scalar_mul(
                    och[:, j, :], ech[:, j, :], col
                )
            if j == CH - 1:
                try:
                    LAST_GATE_INST[(k, ci)] = ins.ins.name
                except Exception:
                    LAST_GATE_INST[(k, ci)] = None
        st.ready_order.append((k, ci))


def _emit_pipeline(st, load_and_score):
    nc = st.nc
    st.ready_order = []

    A, Bg = 0, 1
    ALO, AHI = GROUP_CHUNKS[A]
    BLO, BHI = GROUP_CHUNKS[Bg]

    # group A streams in; DVE-heavy scoring (DVE otherwise idle here)
    load_and_score(A, dve_tiles=5, pe_tiles=3)
    st.deinterleave(A)
    # A prefix + thr32 + k32 masks; first chunks gated on DVE (lowest
    # latency to the first store), rest on ACT
    _prefix(st, A)
    thr = _rounds(st, A, 4)
    difA32 = _dif(st, A, 32, thr)
    _sig_and_masks(st, A, 32, difA32)
    _gate(st, 32, range(ALO, ALO + 2), "vector")
    # A chain to thr64/thr128 (pure DVE): emitted before B's load so the
    # rounds win DVE priority ties against B's score ops — the k64/k128
    # masks are what feeds the store pipe in the 33-50us window
    thr64 = _rounds(st, A, 8)
    thr128 = _rounds(st, A, 16)
    difA64 = _dif(st, A, 64, thr64)
    difA128 = _dif(st, A, 128, thr128)
    # group B streams in; scoring split DVE/Pool
    load_and_score(Bg, dve_tiles=4, pe_tiles=2)
    st.deinterleave(Bg)
    _gate(st, 32, range(ALO + 2, AHI), "scalar")
    _sig_and_masks(st, A, 64, difA64)
    _sig_and_masks(st, A, 128, difA128)
    # ACT gates the first k64 chunks while DVE runs B's chain
    _gate(st, 64, range(ALO, ALO + 2), "scalar")
    _gate(st, 128, range(ALO, AHI), "gpsimd")
    # B prefix + thr32 + k32 masks
    _prefix(st, Bg)
    thr = _rounds(st, Bg, 4)
    difB32 = _dif(st, Bg, 32, thr)
    _sig_and_masks(st, Bg, 32, difB32)
    _gate(st, 32, range(BLO, BLO + 4), "scalar")
    # B chain to thr64/128 (DVE)
    thr64b = _rounds(st, Bg, 8)
    thr128b = _rounds(st, Bg, 16)
    difB64 = _dif(st, Bg, 64, thr64b)
    difB128 = _dif(st, Bg, 128, thr128b)
    _sig_and_masks(st, Bg, 64, difB64)
    _sig_and_masks(st, Bg, 128, difB128)
    _gate(st, 32, range(BLO + 4, BHI), "scalar")
    # remaining k64 on DVE; k128 on Pool
    _gate(st, 64, range(ALO + 2, AHI), "vector")
    _gate(st, 64, range(BLO, BHI), "vector")
    _gate(st, 128, range(BLO, BHI), "gpsimd")

    # stores ordered by estimated production time (us) — the single SP
    # sequencer issues in emission order, so a store emitted too early
    # would stall later-ready streams behind it.
    est = dict(STORE_EST)
    for key in list(st.ochs):
        est.setdefault(key, 60.0)
    for k, ci in sorted(st.ochs, key=lambda kc: est[kc]):
        nc.sync.dma_start(
            out=st.out_t[KS.index(k), :, ci * CH:(ci + 1) * CH, :],
            in_=st.ochs[(k, ci)],
        )


_NC = None


def _get_nc():
    global _NC
    if _NC is None:
        _NC = build_bass()
    return _NC


def kernel(embeddings, w, b, temperature, mask):
    nc = _get_nc()
    embeddings = np.asarray(embeddings, dtype=np.float32)
    w = np.ascontiguousarray(np.asarray(w, dtype=np.float32))
    temperature = np.ascontiguousarray(np.asarray(temperature, dtype=np.float32))
    mask_u8 = np.asarray(mask).astype(np.uint8)

    in_maps = []
    for c in range(N_CORES):
        sl = slice(c * R, (c + 1) * R)
        in_maps.append({
            "embeddings": np.ascontiguousarray(embeddings[sl]),
            "w": w,
            "temperature": temperature,
            "mask": np.ascontiguousarray(mask_u8[sl]),
        })
    res = run_bass_kernel_spmd(nc, in_maps, core_ids=list(range(N_CORES)))
    return np.concatenate([r["out"] for r in res.results], axis=1)



# revision 31
# speedup vs baseline: 1.3190x; 1.3190x over previous
"""Matryoshka soft-top-k gating kernel for Trainium2 (Bass/Tile).

Computes, for each matryoshka scale k in (128, 64, 32):
    scores  = emb @ w  (+ b, which cancels in scores - threshold)
    scores  = where(mask, scores, -BIG)
    thr_k   = k-th largest score per row
    diff    = min(scores - thr_k, CLAMP)       (lower clip can't bind;
              masked -BIG diffs saturate the sigmoid to exactly 0, so the
              post-sigmoid mask multiply is folded away)
    gate    = sigmoid(diff * temperature)
    out_k   = emb * gate[..., None]

Sharding: data-parallel over the batch axis across 8 NeuronCores
(64 rows per core); w/temperature replicated, mask sharded with batch.

HBM traffic is the roofline, so outputs are stored as bf16 (rel err
~4e-3, tolerance 2e-2) in a [3, 128, NT*D] device layout whose innermost
contiguous runs are 8KB (full DMA bandwidth; <512B descriptors cost 2x).
The host pre-transposes embeddings into token-major [128, NT*D] and
reassembles the bf16 outputs.

Per-core pipeline: rows are two 32-row scopes, each with its own 16-round
max8/match_replace chain (chain cost depends only on the 256-wide free
dim, not the partition count).  Scope A (rows 0-31) is scored on
DVE/Pool while its chunks stream in, and its chain runs inside the load
window, so A's six stores (k=32/64/128 x 2 units) flow the moment the
input finishes loading.  Scope B (rows 32-63) is scored on PE
(transpose -> batched ACT copy -> matvec) to keep DVE/Pool free for
gating, and its chain+stores fill the back half.  Gating is spread over
DVE (8-tile tensor_tensor against interleaved gate-column broadcasts),
Pool (same) and ACT (per-tile activation with a scale column).
"""

import numpy as np

import concourse.bacc as bacc
import concourse.bass as bass
import concourse.mybir as mybir
import concourse.tile as tile
from concourse.bass_utils import run_bass_kernel_spmd

N_CORES = 8
B, T, D = 512, 256, 128
R = B // N_CORES          # rows (documents) per core
KS = (128, 64, 32)
CLAMP = 50.0
BIG = 3.4e38              # stands in for -inf in masked_fill
REPL = -3.0e38            # match_replace sentinel (> -BIG)
NT = R * T // 128         # 128-token tiles per core (128)
CH = 16                   # tiles per DMA chunk
NCH = NT // CH            # 8 chunks
UNIT_ROWS = 16            # rows per store unit
UNIT_TILES = UNIT_ROWS * 2          # 32 tiles per store unit
N_UNITS = R // UNIT_ROWS            # 4

# ---- variant switch -------------------------------------------------------
# "f32": embeddings uploaded f32 (safe, ~4e-3 rel err)
# "f16": embeddings uploaded fp16 pre-scaled by 2^10 (~1.4e-2 rel err)
VARIANT = "f16"

f32 = mybir.dt.float32
f16 = mybir.dt.float16
bf16 = mybir.dt.bfloat16
u8 = mybir.dt.uint8
Alu = mybir.AluOpType
Act = mybir.ActivationFunctionType

if VARIANT == "f32":
    EMB_DT = f32
    EMB_NP = np.float32
    EMB_SCALE = 1.0
else:
    EMB_DT = f16
    EMB_NP = np.float16
    EMB_SCALE = 1024.0

# chunk scores: n on DVE, rest on Pool
SCORE_DVE = 5

# Chains: a round costs free-size cycles only (independent of partition
# count), so two cheap 8-round chains on rows 0-15 / 16-31 run inside the
# otherwise-idle load window and feed the first four stores, while the
# single full-rows 16-round chain (starting the moment the last chunk is
# scored) supplies everything else.
SCOPES = {
    "A": (0, 16, 8),
    "B": (16, 32, 8),
    "F": (0, 64, 16),
}
# store plan: ordered (k, unit, scope) — scope supplies the threshold.
STORE_PLAN = [
    (32, 0, "A"), (32, 1, "B"),
    (64, 0, "A"), (64, 1, "B"),
    (32, 2, "F"), (32, 3, "F"),
    (64, 2, "F"), (64, 3, "F"),
    (128, 0, "F"), (128, 1, "F"), (128, 2, "F"), (128, 3, "F"),
]  # slots run ~2.9us apart once the input load drains
# engine pattern per store unit: 4 groups of 8 tiles -> D(VE tt8/tt16),
# P(ool tt8), A(CT per-tile).  Adjacent D groups merge into one wide op.
# A-side rides Pool (idle early), mid units Pool+ACT during the F-chain,
# k=128 units ride DVE (free after the chain).
UNIT_ENGINES = {
    (32, 0): "DDDD", (32, 1): "DDDD",
    (64, 0): "PPAA", (64, 1): "PAPA",
    (32, 2): "PAPA", (32, 3): "APAP",
    (64, 2): "PAPA", (64, 3): "APPA",
    (128, 0): "DDDD", (128, 1): "DDDD", (128, 2): "DDDA", (128, 3): "DDDA",
}


def build_bass():
    nc = bacc.Bacc("TRN2", target_bir_lowering=False, debug=False)

    emb = nc.dram_tensor("emb_tm", [128, NT * D], EMB_DT, kind="ExternalInput")
    w = nc.dram_tensor("w", [D], f32, kind="ExternalInput")
    temp = nc.dram_tensor("temperature", [1], f32, kind="ExternalInput")
    mask = nc.dram_tensor("mask", [R, T], u8, kind="ExternalInput")
    out = nc.dram_tensor("out", [len(KS), 128, NT * D], bf16,
                         kind="ExternalOutput")
    ident_d = nc.inline_tensor(np.eye(128, dtype=np.float32), name="ident128")

    with tile.TileContext(nc) as tc:
        with (
            tc.tile_pool(name="singles", bufs=1) as singles,
            tc.tile_pool(name="out0", bufs=8) as opool,
            tc.tile_pool(name="ebt", bufs=3) as ebtpool,
            tc.tile_pool(name="psum", bufs=2, space="PSUM") as psum,
        ):
            st = _State(nc, singles, psum, opool, ebtpool, out)

            # ---- input loads first so DMA ramps immediately ----
            embbuf = singles.tile([128, NT * D], EMB_DT)
            st.embbuf = embbuf

            def load_chunk(ci):
                nc.sync.dma_start(
                    out=embbuf[:, ci * CH * D:(ci + 1) * CH * D],
                    in_=emb.ap()[:, ci * CH * D:(ci + 1) * CH * D],
                )

            ident = singles.tile([128, 128], f32)
            nc.sync.dma_start(out=ident, in_=ident_d.ap())
            st.ident = ident
            load_chunk(0)
            w_stage = singles.tile([128, D], f32)
            nc.sync.dma_start(out=w_stage, in_=_bcast(w.ap(), 128))
            load_chunk(1)
            w_col_stage = singles.tile([128, 1], f32)
            nc.sync.dma_start(
                out=w_col_stage,
                in_=bass.AP(tensor=w.ap().tensor, offset=0,
                            ap=[[1, 128], [0, 1]]),
            )
            w_col = singles.tile([128, 1], EMB_DT)
            nc.vector.tensor_copy(w_col, w_col_stage)
            st.w_col = w_col
            load_chunk(2)
            for s, (lo, hi, _) in SCOPES.items():
                rg = hi - lo
                tcs = singles.tile([rg, 1], f32, tag=f"tc{s}", name=f"tc{s}")
                nc.sync.dma_start(out=tcs, in_=_bcast(temp.ap(), rg))
                st.temp_col[s] = tcs
            for s, (lo, hi, _) in SCOPES.items():
                rg = hi - lo
                mus = singles.tile([rg, T], u8, tag=f"mu{s}", name=f"mu{s}")
                nc.sync.dma_start(out=mus, in_=mask.ap()[lo:hi, :])
                st.mask_u[s] = mus
            for ci in range(3, NCH):
                load_chunk(ci)

            # ---- SBUF constants ----
            w_rep = singles.tile([128, D], EMB_DT)
            nc.vector.tensor_copy(w_rep, w_stage)
            st.w_rep = w_rep
            if EMB_DT is not f32:
                ident16 = singles.tile([128, 128], EMB_DT)
                nc.vector.tensor_copy(ident16, ident)
                st.ident16 = ident16
            else:
                st.ident16 = ident
            if EMB_SCALE != 1.0:
                for s in SCOPES:
                    nc.gpsimd.tensor_scalar_mul(
                        st.temp_col[s], st.temp_col[s], 1.0 / EMB_SCALE)
            sig_warm = singles.tile([1, 1], f32)
            nc.scalar.activation(sig_warm, st.temp_col["A"][0:1, :],
                                 Act.Sigmoid, bias=0.0, scale=1.0)

            # PE p-state warmup: keep PE busy before the first chunk lands
            # so transposes run at full clock (p-state HIGH needs ~3us).
            pwarm = st.psum.tile([128, 128], f32, tag="pst", name="pwarm",
                                 bufs=2)
            for _ in range(10):
                nc.tensor.transpose(pwarm, ident, ident)

            st.scores_tm = singles.tile([128, NT], f32)
            st.trash_d = singles.tile([128, 1], EMB_DT)
            st.trash_p = singles.tile([128, 1], EMB_DT)

            _emit_pipeline(st)

    nc.compile()
    return nc


def _bcast(ap, n):
    """Replicate a DRAM vector across n partitions (0-step leading dim)."""
    return bass.AP(tensor=ap.tensor, offset=ap.offset, ap=[[0, n]] + list(ap.ap))


class _State:
    def __init__(self, nc, singles, psum, opool, ebtpool, out):
        self.nc = nc
        self.singles = singles
        self.psum = psum
        self.opool = opool
        self.ebtpool = ebtpool
        self.out = out
        self.embbuf = None
        self.ident = None
        self.ident16 = None
        self.w_rep = None
        self.w_col = None
        self.mask_u = {}
        self.temp_col = {}
        self.mask_f = {}
        self.fillt = {}
        self.scores_tm = None
        self.trash_d = None
        self.trash_p = None
        self.scores_e = None
        self.scores_o = None
        self.srm = {}
        self.work = {}
        self.rounds_done = {}
        self.last_mx = {}
        self.gcols = {}
        self.ebts = {}
        self.ochs = {}

    def mark(self, label):
        PHASES.append((label, self.nc.next_id()))

    def tile_ap(self, t):
        return self.embbuf[:, t * D:(t + 1) * D]

    def och_ap(self, och, t_local, n):
        return och[:, t_local * D:(t_local + n) * D]


NPE = 12  # tiles per chunk scored on PE; the rest go to Pool STT


def _score_transpose(st, ci):
    """PE-transpose 6 of the chunk's tiles to PSUM, start the ebT copy,
    and score the remaining 2 tiles with Pool fused multiply+reduce."""
    st.mark(f'scoreT_c{ci}')
    nc = st.nc
    ptile = st.psum.tile([128, NPE * 128], EMB_DT, tag="ptile", name=f"pt{ci}",
                         bufs=2)
    for j in range(NPE):
        t = ci * CH + j
        nc.tensor.transpose(ptile[:, j * 128:(j + 1) * 128], st.tile_ap(t),
                            st.ident16)
    ebT = st.ebtpool.tile([128, NPE * 128], EMB_DT, tag="ebT")
    nc.scalar.copy(ebT, ptile)
    st.ebts[ci] = ebT
    for j in range(NPE, CH):
        t = ci * CH + j
        nc.vector.scalar_tensor_tensor(
            out=st.trash_d.broadcast_to([128, D]),
            in0=st.tile_ap(t), scalar=1.0, in1=st.w_rep,
            op0=Alu.mult, op1=Alu.mult,
            accum_out=st.scores_tm[:, t:t + 1],
        )


def _score_matvec(st, ci):
    """Matvec a transposed chunk against w_col; copy the score columns out."""
    st.mark(f'scoreM_c{ci}')
    nc = st.nc
    ebT = st.ebts[ci]
    psc = st.psum.tile([128, NPE], f32, tag="psc", name=f"psc{ci}", bufs=2)
    for j in range(NPE):
        nc.tensor.matmul(psc[:, j:j + 1], ebT[:, j * 128:(j + 1) * 128],
                         st.w_col)
    nc.vector.tensor_copy(st.scores_tm[:, ci * CH:ci * CH + NPE], psc)


def _score_chunk(st, ci):
    """Software-pipelined: transposes of chunk ci, matvecs of chunk ci-1 —
    one chunk of lag absorbs the PE->ACT->PE copy latency."""
    _score_transpose(st, ci)
    if ci >= 1:
        _score_matvec(st, ci - 1)
    if ci == NCH - 1:
        _score_matvec(st, ci)


def _deinterleave(st, row_lo, row_hi):
    st.mark(f'deint_{row_lo}_{row_hi}')
    nc = st.nc
    if st.scores_e is None:
        st.scores_e = st.singles.tile([128, R], f32)
        st.scores_o = st.singles.tile([128, R], f32)
    n = row_hi - row_lo
    src = st.scores_tm
    se = bass.AP(tensor=src.tensor, offset=src.offset + 2 * row_lo,
                 ap=[list(src.ap[0]), [2, n]])
    so = bass.AP(tensor=src.tensor, offset=src.offset + 2 * row_lo + 1,
                 ap=[list(src.ap[0]), [2, n]])
    nc.vector.tensor_copy(st.scores_e[:, row_lo:row_hi], se)
    nc.vector.tensor_copy(st.scores_o[:, row_lo:row_hi], so)


def _prefix(st, scope):
    """Masked row-major scores for a scope: transpose + mask fill.
    The scope's mask/fill tiles convert just-in-time on Pool."""
    st.mark(f'prefix_{scope}')
    nc = st.nc
    lo, hi, _ = SCOPES[scope]
    rg = hi - lo
    mfs = st.singles.tile([rg, T], f32, tag=f"mf{scope}", name=f"mf{scope}")
    nc.gpsimd.tensor_copy(mfs, st.mask_u[scope])
    ft = st.singles.tile([rg, T], f32, tag=f"ft{scope}", name=f"ft{scope}")
    nc.gpsimd.tensor_scalar(
        out=ft, in0=mfs, scalar1=1.0, scalar2=BIG,
        op0=Alu.subtract, op1=Alu.mult,
    )
    st.mask_f[scope] = mfs
    st.fillt[scope] = ft
    srm = st.singles.tile([rg, T], f32, tag=f"srm{scope}", name=f"srm{scope}")
    pse = st.psum.tile([rg, 128], f32, tag="pst", name=f"pse{scope}", bufs=2)
    nc.tensor.transpose(pse, st.scores_e[:, lo:hi], st.ident)
    nc.vector.tensor_copy(srm[:, 0:128], pse)
    pso = st.psum.tile([rg, 128], f32, tag="pst", name=f"pso{scope}", bufs=2)
    nc.tensor.transpose(pso, st.scores_o[:, lo:hi], st.ident)
    nc.vector.tensor_copy(srm[:, 128:256], pso)
    msk = st.singles.tile([rg, T], f32, tag=f"smsk{scope}",
                          name=f"smsk{scope}")
    nc.vector.scalar_tensor_tensor(
        out=msk, in0=srm, scalar=1.0, in1=st.mask_f[scope],
        op0=Alu.mult, op1=Alu.mult,
    )
    nc.vector.tensor_add(msk, msk, st.fillt[scope])
    st.srm[scope] = msk
    st.work[scope] = st.singles.tile([rg, T], f32, tag=f"work{scope}",
                                     name=f"work{scope}")
    st.rounds_done[scope] = 0


def _rounds(st, scope, upto):
    st.mark(f'rounds_{scope}_{upto}')
    nc = st.nc
    lo, hi, n_rounds = SCOPES[scope]
    rg = hi - lo
    while st.rounds_done[scope] < upto:
        r = st.rounds_done[scope]
        mx = st.singles.tile([rg, 8], f32, tag=f"mx{scope}_{r}",
                             name=f"mx{scope}_{r}")
        src = st.srm[scope] if r == 0 else st.work[scope]
        nc.vector.max(out=mx, in_=src)
        if r < n_rounds - 1:
            nc.vector.match_replace(
                out=st.work[scope], in_to_replace=mx, in_values=src,
                imm_value=REPL,
            )
        st.last_mx[scope] = mx
        st.rounds_done[scope] += 1
    return st.last_mx[scope][:, 7:8]


def _make_gcols(st, scope, k, thr):
    """diff -> sigmoid -> transpose into interleaved gate cols [128, 2*rg]."""
    st.mark(f'gcols_{scope}_{k}')
    nc = st.nc
    lo, hi, _ = SCOPES[scope]
    rg = hi - lo
    dif = st.singles.tile([rg, T], f32, tag=f"dif{scope}_{k}",
                          name=f"dif{scope}_{k}")
    nc.vector.tensor_scalar(
        out=dif, in0=st.srm[scope], scalar1=thr, scalar2=CLAMP * EMB_SCALE,
        op0=Alu.subtract, op1=Alu.min,
    )
    nc.scalar.activation(dif, dif, Act.Sigmoid, bias=0.0,
                         scale=st.temp_col[scope])
    g = st.singles.tile([128, 2 * rg], f32, tag=f"g{scope}_{k}",
                        name=f"g{scope}_{k}")
    pme = st.psum.tile([128, rg], f32, tag="pst", name=f"pme{scope}{k}",
                       bufs=2)
    nc.tensor.transpose(pme, dif[:, 0:128], st.ident[:rg, :rg])
    ge = bass.AP(tensor=g.tensor, offset=g.offset, ap=[list(g.ap[0]), [2, rg]])
    nc.vector.tensor_copy(ge, pme)
    pmo = st.psum.tile([128, rg], f32, tag="pst", name=f"pmo{scope}{k}",
                       bufs=2)
    nc.tensor.transpose(pmo, dif[:, 128:256], st.ident[:rg, :rg])
    go = bass.AP(tensor=g.tensor, offset=g.offset + 1,
                 ap=[list(g.ap[0]), [2, rg]])
    nc.vector.tensor_copy(go, pmo)
    st.gcols[(scope, k)] = g


def _gate_unit(st, k, unit, scope):
    st.mark(f'gate_{k}_{unit}')
    nc = st.nc
    lo, hi, _ = SCOPES[scope]
    g = st.gcols[(scope, k)]
    och = st.opool.tile([128, UNIT_TILES * D], bf16, tag="och")
    st.ochs[(k, unit)] = och
    t0 = unit * UNIT_TILES
    pat = UNIT_ENGINES[(k, unit)]
    gi = 0
    while gi < 4:
        eng = pat[gi]
        ngr = 1
        if eng == "D":
            while gi + ngr < 4 and pat[gi + ngr] == eng:
                ngr += 1
        ts = t0 + gi * 8
        gofs = ts - 2 * lo
        if eng == "A":
            for j in range(8):
                t = ts + j
                col = g[:, gofs + j:gofs + j + 1]
                nc.scalar.activation(
                    st.och_ap(och, t - t0, 1), st.tile_ap(t), Act.Copy,
                    bias=0.0, scale=col,
                )
        else:
            n = ngr * 8
            gb = bass.AP(tensor=g.tensor, offset=g.offset + gofs,
                         ap=[list(g.ap[0]), [1, n], [0, D]])
            e = nc.vector if eng == "D" else nc.gpsimd
            e.tensor_tensor(
                out=st.och_ap(och, ts - t0, n),
                in0=st.embbuf[:, ts * D:(ts + n) * D],
                in1=gb, op=Alu.mult,
            )
        gi += ngr


def _store_unit(st, k, unit):
    st.mark(f'store_{k}_{unit}')
    nc = st.nc
    k_i = KS.index(k)
    nc.sync.dma_start(
        out=st.out.ap()[k_i, :,
                        unit * UNIT_TILES * D:(unit + 1) * UNIT_TILES * D],
        in_=st.ochs[(k, unit)],
    )


def _emit_pipeline(st):
    _score_chunk(st, 0)
    _score_chunk(st, 1)
    _score_chunk(st, 2)
    # scope A (rows 0-15 = chunks 0-1)
    _deinterleave(st, 0, 16)
    _prefix(st, "A")
    thrA32 = _rounds(st, "A", 4)
    _make_gcols(st, "A", 32, thrA32)
    _score_chunk(st, 3)
    _score_chunk(st, 4)
    # scope B (rows 16-31 = chunks 2-3)
    _deinterleave(st, 16, 32)
    _prefix(st, "B")
    thrB32 = _rounds(st, "B", 4)
    _make_gcols(st, "B", 32, thrB32)
    _gate_unit(st, 32, 0, "A")
    _store_unit(st, 32, 0)
    _score_chunk(st, 5)
    _gate_unit(st, 32, 1, "B")
    _store_unit(st, 32, 1)
    _score_chunk(st, 6)
    thrA64 = _rounds(st, "A", 8)
    _make_gcols(st, "A", 64, thrA64)
    _gate_unit(st, 64, 0, "A")
    _store_unit(st, 64, 0)
    _score_chunk(st, 7)
    thrB64 = _rounds(st, "B", 8)
    _make_gcols(st, "B", 64, thrB64)
    _gate_unit(st, 64, 1, "B")
    _store_unit(st, 64, 1)
    # full chain over all 64 rows supplies every remaining threshold
    _deinterleave(st, 32, 64)
    _prefix(st, "F")
    thrF32 = _rounds(st, "F", 4)
    _make_gcols(st, "F", 32, thrF32)
    _gate_unit(st, 32, 2, "F")
    _store_unit(st, 32, 2)
    _gate_unit(st, 32, 3, "F")
    _store_unit(st, 32, 3)
    thrF64 = _rounds(st, "F", 8)
    _make_gcols(st, "F", 64, thrF64)
    _gate_unit(st, 64, 2, "F")
    _store_unit(st, 64, 2)
    _gate_unit(st, 64, 3, "F")
    _store_unit(st, 64, 3)
    thrF128 = _rounds(st, "F", 16)
    _make_gcols(st, "F", 128, thrF128)
    for u in range(4):
        _gate_unit(st, 128, u, "F")
        _store_unit(st, 128, u)


PHASES = []


_NC = None


def _get_nc():
    global _NC
    if _NC is None:
        _NC = build_bass()
    return _NC


def make_in_maps(embeddings, w, temperature, mask):
    """Shard + device-layout the full inputs for the 8 cores."""
    emb = np.asarray(embeddings, dtype=np.float32)
    w = np.ascontiguousarray(np.asarray(w, dtype=np.float32))
    temp = np.ascontiguousarray(np.asarray(temperature, dtype=np.float32))
    mask_u8 = np.asarray(mask).astype(np.uint8)
    in_maps = []
    for c in range(N_CORES):
        sl = slice(c * R, (c + 1) * R)
        esh = emb[sl].reshape(NT, 128, D).transpose(1, 0, 2).reshape(128, NT * D)
        if EMB_SCALE != 1.0:
            esh = esh * EMB_SCALE
        in_maps.append({
            "emb_tm": np.ascontiguousarray(esh.astype(EMB_NP)),
            "w": w,
            "temperature": temp,
            "mask": np.ascontiguousarray(mask_u8[sl]),
        })
    return in_maps


def postprocess(results):
    """Device bf16 [3, 128, NT*D] outputs -> full [3, B, T, D] f32."""
    outs = []
    for r in results:
        o = np.asarray(r["out"]).astype(np.float32)
        if EMB_SCALE != 1.0:
            o *= 1.0 / EMB_SCALE
        o = o.reshape(len(KS), 128, NT, D).transpose(0, 2, 1, 3)
        outs.append(o.reshape(len(KS), R, T, D))
    return np.concatenate(outs, axis=1)


def kernel(embeddings, w, b, temperature, mask):
    nc = _get_nc()
    in_maps = make_in_maps(embeddings, w, temperature, mask)
    res = run_bass_kernel_spmd(nc, in_maps, core_ids=list(range(N_CORES)))
    return postprocess(res.results)


# revision 36
# speedup vs baseline: 1.4362x; 1.0889x over previous
"""Matryoshka soft-top-k gating kernel for Trainium2 (Bass/Tile).

Computes, for each matryoshka scale k in (128, 64, 32):
    scores  = emb @ w  (+ b, which cancels in scores - threshold)
    scores  = where(mask, scores, -BIG)
    thr_k   = k-th largest score per row
    diff    = min(scores - thr_k, CLAMP)       (lower clip can't bind;
              masked -BIG diffs saturate the sigmoid to exactly 0, so the
              post-sigmoid mask multiply is folded away)
    gate    = sigmoid(diff * temperature)
    out_k   = emb * gate[..., None]

Sharding: data-parallel over the batch axis across 8 NeuronCores
(64 rows per core); w/temperature replicated, mask sharded with batch.

HBM traffic is the roofline, so outputs are stored as bf16 (rel err
~4e-3, tolerance 2e-2) in a [3, 128, NT*D] device layout whose innermost
contiguous runs are 8KB (full DMA bandwidth; <512B descriptors cost 2x).
The host pre-transposes embeddings into token-major [128, NT*D] and
reassembles the bf16 outputs.

Per-core pipeline: rows are two 32-row scopes, each with its own 16-round
max8/match_replace chain (chain cost depends only on the 256-wide free
dim, not the partition count).  Scope A (rows 0-31) is scored on
DVE/Pool while its chunks stream in, and its chain runs inside the load
window, so A's six stores (k=32/64/128 x 2 units) flow the moment the
input finishes loading.  Scope B (rows 32-63) is scored on PE
(transpose -> batched ACT copy -> matvec) to keep DVE/Pool free for
gating, and its chain+stores fill the back half.  Gating is spread over
DVE (8-tile tensor_tensor against interleaved gate-column broadcasts),
Pool (same) and ACT (per-tile activation with a scale column).
"""

import numpy as np

import concourse.bacc as bacc
import concourse.bass as bass
import concourse.mybir as mybir
import concourse.tile as tile
from concourse.bass_utils import run_bass_kernel_spmd

N_CORES = 8
B, T, D = 512, 256, 128
R = B // N_CORES          # rows (documents) per core
KS = (128, 64, 32)
CLAMP = 50.0
BIG = 3.4e38              # stands in for -inf in masked_fill
REPL = -3.0e38            # match_replace sentinel (> -BIG)
NT = R * T // 128         # 128-token tiles per core (128)
CH = 16                   # tiles per DMA chunk
NCH = NT // CH            # 8 chunks
UNIT_ROWS = 16            # rows per store unit
UNIT_TILES = UNIT_ROWS * 2          # 32 tiles per store unit
N_UNITS = R // UNIT_ROWS            # 4

# ---- variant switch -------------------------------------------------------
# "f32": embeddings uploaded f32 (safe, ~4e-3 rel err)
# "f16": embeddings uploaded fp16 pre-scaled by 2^10 (~1.4e-2 rel err)
VARIANT = "f16"

f32 = mybir.dt.float32
f16 = mybir.dt.float16
bf16 = mybir.dt.bfloat16
u8 = mybir.dt.uint8
Alu = mybir.AluOpType
Act = mybir.ActivationFunctionType

if VARIANT == "f32":
    EMB_DT = f32
    EMB_NP = np.float32
    EMB_SCALE = 1.0
else:
    EMB_DT = f16
    EMB_NP = np.float16
    EMB_SCALE = 1024.0

# chunk scores: n on DVE, rest on Pool
SCORE_DVE = 5

# Chains: a round costs free-size cycles only (independent of partition
# count), so two cheap 8-round chains on rows 0-15 / 16-31 run inside the
# otherwise-idle load window and feed the first four stores, while the
# single full-rows 16-round chain (starting the moment the last chunk is
# scored) supplies everything else.
SCOPES = {
    "A": (0, 16, 8),
    "B": (16, 32, 8),
    "F": (0, 64, 16),
}
# store plan: ordered (k, unit, scope) — scope supplies the threshold.
STORE_PLAN = [
    (32, 0, "A"), (32, 1, "B"),
    (64, 0, "A"), (64, 1, "B"),
    (32, 2, "F"), (32, 3, "F"),
    (64, 2, "F"), (64, 3, "F"),
    (128, 0, "F"), (128, 1, "F"), (128, 2, "F"), (128, 3, "F"),
]  # slots run ~2.9us apart once the input load drains
# engine pattern per store unit: 4 groups of 8 tiles -> D(VE tt8/tt16),
# P(ool tt8), A(CT per-tile).  Adjacent D groups merge into one wide op.
# A-side rides Pool (idle early), mid units Pool+ACT during the F-chain,
# k=128 units ride DVE (free after the chain).
UNIT_ENGINES = {
    (32, 0): "PAPA", (32, 1): "APAP",
    (64, 0): "PPAA", (64, 1): "PAPA",
    (32, 2): "PAPA", (32, 3): "APAP",
    (64, 2): "PAPA", (64, 3): "APPA",
    (128, 0): "DDDD", (128, 1): "DDDD", (128, 2): "DDPA", (128, 3): "DPAA",
}


def build_bass():
    nc = bacc.Bacc("TRN2", target_bir_lowering=False, debug=False)

    emb = nc.dram_tensor("emb_tm", [128, NT * D], EMB_DT, kind="ExternalInput")
    w = nc.dram_tensor("w", [D], f32, kind="ExternalInput")
    temp = nc.dram_tensor("temperature", [1], f32, kind="ExternalInput")
    mask = nc.dram_tensor("mask", [R, T], u8, kind="ExternalInput")
    out = nc.dram_tensor("out", [len(KS), 128, NT * D], bf16,
                         kind="ExternalOutput")
    ident_d = nc.inline_tensor(np.eye(128, dtype=np.float32), name="ident128")

    with tile.TileContext(nc) as tc:
        with (
            tc.tile_pool(name="singles", bufs=1) as singles,
            tc.tile_pool(name="out0", bufs=8) as opool,
            tc.tile_pool(name="ebt", bufs=3) as ebtpool,
            tc.tile_pool(name="psum", bufs=2, space="PSUM") as psum,
        ):
            st = _State(nc, singles, psum, opool, ebtpool, out)

            # ---- input loads first so DMA ramps immediately ----
            embbuf = singles.tile([128, NT * D], EMB_DT)
            st.embbuf = embbuf

            def load_chunk(ci):
                nc.sync.dma_start(
                    out=embbuf[:, ci * CH * D:(ci + 1) * CH * D],
                    in_=emb.ap()[:, ci * CH * D:(ci + 1) * CH * D],
                )

            ident = singles.tile([128, 128], f32)
            nc.sync.dma_start(out=ident, in_=ident_d.ap())
            st.ident = ident
            load_chunk(0)
            w_stage = singles.tile([128, D], f32)
            nc.sync.dma_start(out=w_stage, in_=_bcast(w.ap(), 128))
            load_chunk(1)
            w_col_stage = singles.tile([128, 1], f32)
            nc.sync.dma_start(
                out=w_col_stage,
                in_=bass.AP(tensor=w.ap().tensor, offset=0,
                            ap=[[1, 128], [0, 1]]),
            )
            w_col = singles.tile([128, 1], EMB_DT)
            nc.vector.tensor_copy(w_col, w_col_stage)
            st.w_col = w_col
            load_chunk(2)
            for s, (lo, hi, _) in SCOPES.items():
                rg = hi - lo
                tcs = singles.tile([rg, 1], f32, tag=f"tc{s}", name=f"tc{s}")
                nc.sync.dma_start(out=tcs, in_=_bcast(temp.ap(), rg))
                st.temp_col[s] = tcs
            for s, (lo, hi, _) in SCOPES.items():
                rg = hi - lo
                mus = singles.tile([rg, T], u8, tag=f"mu{s}", name=f"mu{s}")
                nc.sync.dma_start(out=mus, in_=mask.ap()[lo:hi, :])
                st.mask_u[s] = mus
            for ci in range(3, NCH):
                load_chunk(ci)

            # ---- SBUF constants ----
            w_rep = singles.tile([128, D], EMB_DT)
            nc.vector.tensor_copy(w_rep, w_stage)
            st.w_rep = w_rep
            if EMB_DT is not f32:
                ident16 = singles.tile([128, 128], EMB_DT)
                nc.vector.tensor_copy(ident16, ident)
                st.ident16 = ident16
            else:
                st.ident16 = ident
            if EMB_SCALE != 1.0:
                for s in SCOPES:
                    nc.gpsimd.tensor_scalar_mul(
                        st.temp_col[s], st.temp_col[s], 1.0 / EMB_SCALE)
            sig_warm = singles.tile([1, 1], f32)
            nc.scalar.activation(sig_warm, st.temp_col["A"][0:1, :],
                                 Act.Sigmoid, bias=0.0, scale=1.0)

            # PE p-state warmup: keep PE busy before the first chunk lands
            # so transposes run at full clock (p-state HIGH needs ~3us).
            pwarm = st.psum.tile([128, 128], f32, tag="pst", name="pwarm",
                                 bufs=2)
            for _ in range(10):
                nc.tensor.transpose(pwarm, ident, ident)

            st.scores_tm = singles.tile([128, NT], f32)
            st.trash_d = singles.tile([128, 1], EMB_DT)
            st.trash_p = singles.tile([128, 1], EMB_DT)

            _emit_pipeline(st)

    nc.compile()
    return nc


def _bcast(ap, n):
    """Replicate a DRAM vector across n partitions (0-step leading dim)."""
    return bass.AP(tensor=ap.tensor, offset=ap.offset, ap=[[0, n]] + list(ap.ap))


class _State:
    def __init__(self, nc, singles, psum, opool, ebtpool, out):
        self.nc = nc
        self.singles = singles
        self.psum = psum
        self.opool = opool
        self.ebtpool = ebtpool
        self.out = out
        self.embbuf = None
        self.ident = None
        self.ident16 = None
        self.w_rep = None
        self.w_col = None
        self.mask_u = {}
        self.temp_col = {}
        self.mask_f = {}
        self.fillt = {}
        self.scores_tm = None
        self.trash_d = None
        self.trash_p = None
        self.scores_e = None
        self.scores_o = None
        self.srm = {}
        self.work = {}
        self.rounds_done = {}
        self.last_mx = {}
        self.gcols = {}
        self.ebts = {}
        self.ochs = {}

    def mark(self, label):
        PHASES.append((label, self.nc.next_id()))

    def tile_ap(self, t):
        return self.embbuf[:, t * D:(t + 1) * D]

    def och_ap(self, och, t_local, n):
        return och[:, t_local * D:(t_local + n) * D]


NPE = 16  # tiles per chunk scored on PE


def _score_transpose(st, ci):
    """PE-transpose 6 of the chunk's tiles to PSUM, start the ebT copy,
    and score the remaining 2 tiles with Pool fused multiply+reduce."""
    st.mark(f'scoreT_c{ci}')
    nc = st.nc
    ptile = st.psum.tile([128, NPE * 128], EMB_DT, tag="ptile", name=f"pt{ci}",
                         bufs=2)
    for j in range(NPE):
        t = ci * CH + j
        nc.tensor.transpose(ptile[:, j * 128:(j + 1) * 128], st.tile_ap(t),
                            st.ident16)
    ebT = st.ebtpool.tile([128, NPE * 128], EMB_DT, tag="ebT")
    half = NPE * 64
    nc.scalar.copy(ebT[:, 0:half], ptile[:, 0:half])
    nc.vector.tensor_copy(ebT[:, half:], ptile[:, half:])
    st.ebts[ci] = ebT


def _score_matvec(st, ci):
    """Matvec a transposed chunk against w_col; copy the score columns out."""
    st.mark(f'scoreM_c{ci}')
    nc = st.nc
    ebT = st.ebts[ci]
    psc = st.psum.tile([128, NPE], f32, tag="psc", name=f"psc{ci}", bufs=2)
    for j in range(NPE):
        nc.tensor.matmul(psc[:, j:j + 1], ebT[:, j * 128:(j + 1) * 128],
                         st.w_col)
    nc.vector.tensor_copy(st.scores_tm[:, ci * CH:ci * CH + NPE], psc)


def _score_chunk(st, ci):
    """Software-pipelined: transposes of chunk ci, matvecs of chunk ci-1 —
    one chunk of lag absorbs the PE->ACT->PE copy latency."""
    _score_transpose(st, ci)
    if ci >= 1:
        _score_matvec(st, ci - 1)
    if ci == NCH - 1:
        _score_matvec(st, ci)


def _deinterleave(st, row_lo, row_hi):
    st.mark(f'deint_{row_lo}_{row_hi}')
    nc = st.nc
    if st.scores_e is None:
        st.scores_e = st.singles.tile([128, R], f32)
        st.scores_o = st.singles.tile([128, R], f32)
    n = row_hi - row_lo
    src = st.scores_tm
    se = bass.AP(tensor=src.tensor, offset=src.offset + 2 * row_lo,
                 ap=[list(src.ap[0]), [2, n]])
    so = bass.AP(tensor=src.tensor, offset=src.offset + 2 * row_lo + 1,
                 ap=[list(src.ap[0]), [2, n]])
    nc.vector.tensor_copy(st.scores_e[:, row_lo:row_hi], se)
    nc.vector.tensor_copy(st.scores_o[:, row_lo:row_hi], so)


def _prefix(st, scope):
    """Masked row-major scores for a scope: transpose + mask fill.
    The scope's mask/fill tiles convert just-in-time on Pool."""
    st.mark(f'prefix_{scope}')
    nc = st.nc
    lo, hi, _ = SCOPES[scope]
    rg = hi - lo
    mfs = st.singles.tile([rg, T], f32, tag=f"mf{scope}", name=f"mf{scope}")
    nc.gpsimd.tensor_copy(mfs, st.mask_u[scope])
    ft = st.singles.tile([rg, T], f32, tag=f"ft{scope}", name=f"ft{scope}")
    nc.gpsimd.tensor_scalar(
        out=ft, in0=mfs, scalar1=1.0, scalar2=BIG,
        op0=Alu.subtract, op1=Alu.mult,
    )
    st.mask_f[scope] = mfs
    st.fillt[scope] = ft
    srm = st.singles.tile([rg, T], f32, tag=f"srm{scope}", name=f"srm{scope}")
    pse = st.psum.tile([rg, 128], f32, tag="psc", name=f"pse{scope}", bufs=2)
    nc.tensor.transpose(pse, st.scores_e[:, lo:hi], st.ident)
    nc.vector.tensor_copy(srm[:, 0:128], pse)
    pso = st.psum.tile([rg, 128], f32, tag="psc", name=f"pso{scope}", bufs=2)
    nc.tensor.transpose(pso, st.scores_o[:, lo:hi], st.ident)
    nc.vector.tensor_copy(srm[:, 128:256], pso)
    msk = st.singles.tile([rg, T], f32, tag=f"smsk{scope}",
                          name=f"smsk{scope}")
    nc.vector.scalar_tensor_tensor(
        out=msk, in0=srm, scalar=1.0, in1=st.mask_f[scope],
        op0=Alu.mult, op1=Alu.mult,
    )
    nc.vector.tensor_add(msk, msk, st.fillt[scope])
    st.srm[scope] = msk
    st.work[scope] = st.singles.tile([rg, T], f32, tag=f"work{scope}",
                                     name=f"work{scope}")
    st.rounds_done[scope] = 0


def _rounds(st, scope, upto):
    st.mark(f'rounds_{scope}_{upto}')
    nc = st.nc
    lo, hi, n_rounds = SCOPES[scope]
    rg = hi - lo
    while st.rounds_done[scope] < upto:
        r = st.rounds_done[scope]
        mx = st.singles.tile([rg, 8], f32, tag=f"mx{scope}_{r}",
                             name=f"mx{scope}_{r}")
        src = st.srm[scope] if r == 0 else st.work[scope]
        nc.vector.max(out=mx, in_=src)
        if r < n_rounds - 1:
            nc.vector.match_replace(
                out=st.work[scope], in_to_replace=mx, in_values=src,
                imm_value=REPL,
            )
        st.last_mx[scope] = mx
        st.rounds_done[scope] += 1
    return st.last_mx[scope][:, 7:8]


def _make_gcols(st, scope, k, thr):
    """diff -> sigmoid -> transpose into interleaved gate cols [128, 2*rg]."""
    st.mark(f'gcols_{scope}_{k}')
    nc = st.nc
    lo, hi, _ = SCOPES[scope]
    rg = hi - lo
    dif = st.singles.tile([rg, T], f32, tag=f"dif{scope}_{k}",
                          name=f"dif{scope}_{k}")
    nc.vector.tensor_scalar(
        out=dif, in0=st.srm[scope], scalar1=thr, scalar2=CLAMP * EMB_SCALE,
        op0=Alu.subtract, op1=Alu.min,
    )
    nc.scalar.activation(dif, dif, Act.Sigmoid, bias=0.0,
                         scale=st.temp_col[scope])
    g = st.singles.tile([128, 2 * rg], f32, tag=f"g{scope}_{k}",
                        name=f"g{scope}_{k}")
    pme = st.psum.tile([128, rg], f32, tag="pst", name=f"pme{scope}{k}",
                       bufs=2)
    nc.tensor.transpose(pme, dif[:, 0:128], st.ident[:rg, :rg])
    ge = bass.AP(tensor=g.tensor, offset=g.offset, ap=[list(g.ap[0]), [2, rg]])
    nc.vector.tensor_copy(ge, pme)
    pmo = st.psum.tile([128, rg], f32, tag="pst", name=f"pmo{scope}{k}",
                       bufs=2)
    nc.tensor.transpose(pmo, dif[:, 128:256], st.ident[:rg, :rg])
    go = bass.AP(tensor=g.tensor, offset=g.offset + 1,
                 ap=[list(g.ap[0]), [2, rg]])
    nc.vector.tensor_copy(go, pmo)
    st.gcols[(scope, k)] = g


def _gate_unit(st, k, unit, scope):
    st.mark(f'gate_{k}_{unit}')
    nc = st.nc
    lo, hi, _ = SCOPES[scope]
    g = st.gcols[(scope, k)]
    och = st.opool.tile([128, UNIT_TILES * D], bf16, tag="och")
    st.ochs[(k, unit)] = och
    t0 = unit * UNIT_TILES
    pat = UNIT_ENGINES[(k, unit)]
    gi = 0
    while gi < 4:
        eng = pat[gi]
        ngr = 1
        if eng == "D":
            while gi + ngr < 4 and pat[gi + ngr] == eng:
                ngr += 1
        ts = t0 + gi * 8
        gofs = ts - 2 * lo
        if eng == "A":
            for j in range(8):
                t = ts + j
                col = g[:, gofs + j:gofs + j + 1]
                nc.scalar.activation(
                    st.och_ap(och, t - t0, 1), st.tile_ap(t), Act.Copy,
                    bias=0.0, scale=col,
                )
        else:
            n = ngr * 8
            gb = bass.AP(tensor=g.tensor, offset=g.offset + gofs,
                         ap=[list(g.ap[0]), [1, n], [0, D]])
            e = nc.vector if eng == "D" else nc.gpsimd
            e.tensor_tensor(
                out=st.och_ap(och, ts - t0, n),
                in0=st.embbuf[:, ts * D:(ts + n) * D],
                in1=gb, op=Alu.mult,
            )
        gi += ngr


def _store_unit(st, k, unit):
    st.mark(f'store_{k}_{unit}')
    nc = st.nc
    k_i = KS.index(k)
    nc.sync.dma_start(
        out=st.out.ap()[k_i, :,
                        unit * UNIT_TILES * D:(unit + 1) * UNIT_TILES * D],
        in_=st.ochs[(k, unit)],
    )


def _emit_pipeline(st):
    _score_chunk(st, 0)
    _score_chunk(st, 1)
    _score_chunk(st, 2)
    # scope A (rows 0-15 = chunks 0-1)
    _deinterleave(st, 0, 16)
    _prefix(st, "A")
    thrA32 = _rounds(st, "A", 4)
    _make_gcols(st, "A", 32, thrA32)
    _score_chunk(st, 3)
    _score_chunk(st, 4)
    # scope B (rows 16-31 = chunks 2-3)
    _deinterleave(st, 16, 32)
    _prefix(st, "B")
    thrB32 = _rounds(st, "B", 4)
    _make_gcols(st, "B", 32, thrB32)
    _gate_unit(st, 32, 0, "A")
    _store_unit(st, 32, 0)
    _score_chunk(st, 5)
    _gate_unit(st, 32, 1, "B")
    _store_unit(st, 32, 1)
    _score_chunk(st, 6)
    thrA64 = _rounds(st, "A", 8)
    _make_gcols(st, "A", 64, thrA64)
    _gate_unit(st, 64, 0, "A")
    _store_unit(st, 64, 0)
    _score_chunk(st, 7)
    thrB64 = _rounds(st, "B", 8)
    _make_gcols(st, "B", 64, thrB64)
    _gate_unit(st, 64, 1, "B")
    _store_unit(st, 64, 1)
    # full chain over all 64 rows supplies every remaining threshold
    _deinterleave(st, 32, 64)
    _prefix(st, "F")
    thrF32 = _rounds(st, "F", 4)
    _make_gcols(st, "F", 32, thrF32)
    _gate_unit(st, 32, 2, "F")
    _store_unit(st, 32, 2)
    _gate_unit(st, 32, 3, "F")
    _store_unit(st, 32, 3)
    thrF64 = _rounds(st, "F", 8)
    _make_gcols(st, "F", 64, thrF64)
    _gate_unit(st, 64, 2, "F")
    _store_unit(st, 64, 2)
    _gate_unit(st, 64, 3, "F")
    _store_unit(st, 64, 3)
    thrF128 = _rounds(st, "F", 16)
    _make_gcols(st, "F", 128, thrF128)
    for u in range(4):
        _gate_unit(st, 128, u, "F")
        _store_unit(st, 128, u)


PHASES = []


_NC = None


def _get_nc():
    global _NC
    if _NC is None:
        _NC = build_bass()
    return _NC


def make_in_maps(embeddings, w, temperature, mask):
    """Shard + device-layout the full inputs for the 8 cores."""
    emb = np.asarray(embeddings, dtype=np.float32)
    w = np.ascontiguousarray(np.asarray(w, dtype=np.float32))
    temp = np.ascontiguousarray(np.asarray(temperature, dtype=np.float32))
    mask_u8 = np.asarray(mask).astype(np.uint8)
    in_maps = []
    for c in range(N_CORES):
        sl = slice(c * R, (c + 1) * R)
        esh = emb[sl].reshape(NT, 128, D).transpose(1, 0, 2).reshape(128, NT * D)
        if EMB_SCALE != 1.0:
            esh = esh * EMB_SCALE
        in_maps.append({
            "emb_tm": np.ascontiguousarray(esh.astype(EMB_NP)),
            "w": w,
            "temperature": temp,
            "mask": np.ascontiguousarray(mask_u8[sl]),
        })
    return in_maps


def postprocess(results):
    """Device bf16 [3, 128, NT*D] outputs -> full [3, B, T, D] f32."""
    outs = []
    for r in results:
        o = np.asarray(r["out"]).astype(np.float32)
        if EMB_SCALE != 1.0:
            o *= 1.0 / EMB_SCALE
        o = o.reshape(len(KS), 128, NT, D).transpose(0, 2, 1, 3)
        outs.append(o.reshape(len(KS), R, T, D))
    return np.concatenate(outs, axis=1)


def kernel(embeddings, w, b, temperature, mask):
    nc = _get_nc()
    in_maps = make_in_maps(embeddings, w, temperature, mask)
    res = run_bass_kernel_spmd(nc, in_maps, core_ids=list(range(N_CORES)))
    return postprocess(res.results)


# revision 46
# speedup vs baseline: 1.4366x; 1.0003x over previous
"""Matryoshka soft-top-k gating kernel for Trainium2 (Bass/Tile).

Computes, for each matryoshka scale k in (128, 64, 32):
    scores  = emb @ w  (+ b, which cancels in scores - threshold)
    scores  = where(mask, scores, -BIG)
    thr_k   = k-th largest score per row
    diff    = min(scores - thr_k, CLAMP)       (lower clip can't bind;
              masked -BIG diffs saturate the sigmoid to exactly 0, so the
              post-sigmoid mask multiply is folded away)
    gate    = sigmoid(diff * temperature)
    out_k   = emb * gate[..., None]

Sharding: data-parallel over the batch axis across 8 NeuronCores
(64 rows per core); w/temperature replicated, mask sharded with batch.

HBM traffic is the roofline, so embeddings are uploaded as fp16
pre-scaled by 2^10 (the scale dodges the fp16 subnormal cliff; scores
scale out in the sigmoid's temperature and the outputs are unscaled on
the host) and outputs are stored as bf16 (combined rel err ~1.4e-2 vs
the 2e-2 tolerance).  Both live in device layouts whose innermost
contiguous runs are >=2KB: <512B DMA descriptors cost 2x bandwidth.
The host pre-transposes embeddings into token-major [128, NT*D] and
reassembles/unscales the bf16 outputs.

Per-core pipeline: scores are computed on PE (transpose each 128-token
tile against an fp16 identity, batch-copy the transposed chunk to SBUF
split across ACT and DVE, then matvec against w) so DVE stays free for
the serial threshold chains.  A max8+match_replace chain round costs
free-size cycles regardless of partition count, so two narrow 8-round
chains (rows 0-15, 16-31) run inside the load window and feed the first
four stores (k=32/64), while a single full-rows 16-round chain supplies
all remaining thresholds.  Gating is spread over DVE (merged
tensor_tensor groups against interleaved gate-column broadcasts), Pool
and ACT (per-tile activation with an f32 scale column), and the twelve
(k, 16-row) stores are emitted in production order so the store stream
follows the input load with the DMA engines near-continuously busy.
"""

import numpy as np

import concourse.bacc as bacc
import concourse.bass as bass
import concourse.mybir as mybir
import concourse.tile as tile
from concourse.bass_utils import run_bass_kernel_spmd

N_CORES = 8
B, T, D = 512, 256, 128
R = B // N_CORES          # rows (documents) per core
KS = (128, 64, 32)
CLAMP = 50.0
BIG = 3.4e38              # stands in for -inf in masked_fill
REPL = -3.0e38            # match_replace sentinel (> -BIG)
NT = R * T // 128         # 128-token tiles per core (128)
CH = 16                   # tiles per DMA chunk
NCH = NT // CH            # 8 chunks
UNIT_ROWS = 16            # rows per store unit
UNIT_TILES = UNIT_ROWS * 2          # 32 tiles per store unit
N_UNITS = R // UNIT_ROWS            # 4

# ---- variant switch -------------------------------------------------------
# "f32": embeddings uploaded f32 (safe, ~4e-3 rel err)
# "f16": embeddings uploaded fp16 pre-scaled by 2^10 (~1.4e-2 rel err)
VARIANT = "f16"

f32 = mybir.dt.float32
f16 = mybir.dt.float16
bf16 = mybir.dt.bfloat16
u8 = mybir.dt.uint8
Alu = mybir.AluOpType
Act = mybir.ActivationFunctionType

if VARIANT == "f32":
    EMB_DT = f32
    EMB_NP = np.float32
    EMB_SCALE = 1.0
else:
    EMB_DT = f16
    EMB_NP = np.float16
    EMB_SCALE = 1024.0

# chunk scores: n on DVE, rest on Pool
SCORE_DVE = 5

# Chains: a round costs free-size cycles only (independent of partition
# count), so two cheap 8-round chains on rows 0-15 / 16-31 run inside the
# otherwise-idle load window and feed the first four stores, while the
# single full-rows 16-round chain (starting the moment the last chunk is
# scored) supplies everything else.
SCOPES = {
    "A": (0, 16, 8),
    "B": (16, 32, 8),
    "F": (0, 64, 16),
}
# store plan: ordered (k, unit, scope) — scope supplies the threshold.
STORE_PLAN = [
    (32, 0, "A"), (32, 1, "B"),
    (64, 0, "A"), (64, 1, "B"),
    (32, 2, "F"), (32, 3, "F"),
    (64, 2, "F"), (64, 3, "F"),
    (128, 0, "F"), (128, 1, "F"), (128, 2, "F"), (128, 3, "F"),
]  # slots run ~2.9us apart once the input load drains
# engine pattern per store unit: 4 groups of 8 tiles -> D(VE tt8/tt16),
# P(ool tt8), A(CT per-tile).  Adjacent D groups merge into one wide op.
# A-side rides Pool (idle early), mid units Pool+ACT during the F-chain,
# k=128 units ride DVE (free after the chain).
UNIT_ENGINES = {
    (32, 0): "PAPA", (32, 1): "APAP",
    (64, 0): "PPAA", (64, 1): "PAPA",
    (32, 2): "PAPA", (32, 3): "APAP",
    (64, 2): "PAPA", (64, 3): "APPA",
    (128, 0): "DDDD", (128, 1): "DDDD", (128, 2): "DDPA", (128, 3): "DPAA",
}


def build_bass():
    nc = bacc.Bacc("TRN2", target_bir_lowering=False, debug=False)

    emb = nc.dram_tensor("emb_tm", [128, NT * D], EMB_DT, kind="ExternalInput")
    w = nc.dram_tensor("w", [D], f32, kind="ExternalInput")
    temp = nc.dram_tensor("temperature", [1], f32, kind="ExternalInput")
    mask = nc.dram_tensor("mask", [R, T], u8, kind="ExternalInput")
    out = nc.dram_tensor("out", [len(KS), 128, NT * D], bf16,
                         kind="ExternalOutput")
    ident_d = nc.inline_tensor(np.eye(128, dtype=np.float32), name="ident128")

    with tile.TileContext(nc) as tc:
        with (
            tc.tile_pool(name="singles", bufs=1) as singles,
            tc.tile_pool(name="out0", bufs=8) as opool,
            tc.tile_pool(name="ebt", bufs=3) as ebtpool,
            tc.tile_pool(name="psum", bufs=2, space="PSUM") as psum,
        ):
            st = _State(nc, singles, psum, opool, ebtpool, out)

            # ---- input loads first so DMA ramps immediately ----
            embbuf = singles.tile([128, NT * D], EMB_DT)
            st.embbuf = embbuf

            def load_chunk(ci):
                nc.sync.dma_start(
                    out=embbuf[:, ci * CH * D:(ci + 1) * CH * D],
                    in_=emb.ap()[:, ci * CH * D:(ci + 1) * CH * D],
                )

            ident = singles.tile([128, 128], f32)
            nc.sync.dma_start(out=ident, in_=ident_d.ap())
            st.ident = ident
            load_chunk(0)
            w_stage = singles.tile([128, D], f32)
            nc.sync.dma_start(out=w_stage, in_=_bcast(w.ap(), 128))
            load_chunk(1)
            w_col_stage = singles.tile([128, 1], f32)
            nc.sync.dma_start(
                out=w_col_stage,
                in_=bass.AP(tensor=w.ap().tensor, offset=0,
                            ap=[[1, 128], [0, 1]]),
            )
            load_chunk(2)
            for s, (lo, hi, _) in SCOPES.items():
                rg = hi - lo
                mus = singles.tile([rg, T], u8, tag=f"mu{s}", name=f"mu{s}")
                nc.sync.dma_start(out=mus, in_=mask.ap()[lo:hi, :])
                st.mask_u[s] = mus
            for s, (lo, hi, _) in SCOPES.items():
                rg = hi - lo
                tcs = singles.tile([rg, 1], f32, tag=f"tc{s}", name=f"tc{s}")
                nc.sync.dma_start(out=tcs, in_=_bcast(temp.ap(), rg))
                st.temp_col[s] = tcs
            load_chunk(3)
            for ci in range(4, NCH):
                load_chunk(ci)

            # ---- SBUF constants ----
            w_col = singles.tile([128, 1], EMB_DT)
            nc.vector.tensor_copy(w_col, w_col_stage)
            st.w_col = w_col
            w_rep = singles.tile([128, D], EMB_DT)
            nc.vector.tensor_copy(w_rep, w_stage)
            st.w_rep = w_rep
            if EMB_DT is not f32:
                ident16 = singles.tile([128, 128], EMB_DT)
                nc.vector.tensor_copy(ident16, ident)
                st.ident16 = ident16
            else:
                st.ident16 = ident
            if EMB_SCALE != 1.0:
                for s in SCOPES:
                    nc.gpsimd.tensor_scalar_mul(
                        st.temp_col[s], st.temp_col[s], 1.0 / EMB_SCALE)
            sig_warm = singles.tile([1, 1], f32)
            nc.scalar.activation(sig_warm, st.temp_col["A"][0:1, :],
                                 Act.Sigmoid, bias=0.0, scale=1.0)

            # PE p-state warmup: keep PE busy before the first chunk lands
            # so transposes run at full clock (p-state HIGH needs ~3us).
            pwarm = st.psum.tile([128, 128], f32, tag="pst", name="pwarm",
                                 bufs=2)
            for _ in range(10):
                nc.tensor.transpose(pwarm, ident, ident)

            st.scores_tm = singles.tile([128, NT], f32)
            st.trash_d = singles.tile([128, 1], EMB_DT)
            st.trash_p = singles.tile([128, 1], EMB_DT)

            _emit_pipeline(st)

    nc.compile()
    return nc


def _bcast(ap, n):
    """Replicate a DRAM vector across n partitions (0-step leading dim)."""
    return bass.AP(tensor=ap.tensor, offset=ap.offset, ap=[[0, n]] + list(ap.ap))


class _State:
    def __init__(self, nc, singles, psum, opool, ebtpool, out):
        self.nc = nc
        self.singles = singles
        self.psum = psum
        self.opool = opool
        self.ebtpool = ebtpool
        self.out = out
        self.embbuf = None
        self.ident = None
        self.ident16 = None
        self.w_rep = None
        self.w_col = None
        self.mask_u = {}
        self.temp_col = {}
        self.mask_f = {}
        self.fillt = {}
        self.scores_tm = None
        self.trash_d = None
        self.trash_p = None
        self.scores_e = None
        self.scores_o = None
        self.srm = {}
        self.work = {}
        self.rounds_done = {}
        self.last_mx = {}
        self.gcols = {}
        self.ebts = {}
        self.ochs = {}

    def mark(self, label):
        PHASES.append((label, self.nc.next_id()))

    def tile_ap(self, t):
        return self.embbuf[:, t * D:(t + 1) * D]

    def och_ap(self, och, t_local, n):
        return och[:, t_local * D:(t_local + n) * D]


NPE = 16  # tiles per chunk scored on PE


def _score_transpose(st, ci):
    """PE-transpose 6 of the chunk's tiles to PSUM, start the ebT copy,
    and score the remaining 2 tiles with Pool fused multiply+reduce."""
    st.mark(f'scoreT_c{ci}')
    nc = st.nc
    ptile = st.psum.tile([128, NPE * 128], EMB_DT, tag="ptile", name=f"pt{ci}",
                         bufs=2)
    for j in range(NPE):
        t = ci * CH + j
        nc.tensor.transpose(ptile[:, j * 128:(j + 1) * 128], st.tile_ap(t),
                            st.ident16)
    ebT = st.ebtpool.tile([128, NPE * 128], EMB_DT, tag="ebT")
    half = NPE * 64
    nc.scalar.copy(ebT[:, 0:half], ptile[:, 0:half])
    nc.vector.tensor_copy(ebT[:, half:], ptile[:, half:])
    st.ebts[ci] = ebT


def _score_matvec(st, ci):
    """Matvec a transposed chunk against w_col; copy the score columns out."""
    st.mark(f'scoreM_c{ci}')
    nc = st.nc
    ebT = st.ebts[ci]
    psc = st.psum.tile([128, NPE], f32, tag="psc", name=f"psc{ci}", bufs=2)
    for j in range(NPE):
        nc.tensor.matmul(psc[:, j:j + 1], ebT[:, j * 128:(j + 1) * 128],
                         st.w_col)
    nc.vector.tensor_copy(st.scores_tm[:, ci * CH:ci * CH + NPE], psc)


def _score_chunk(st, ci):
    """Software-pipelined: transposes of chunk ci, matvecs of chunk ci-1 —
    one chunk of lag absorbs the PE->ACT->PE copy latency."""
    _score_transpose(st, ci)
    if ci >= 1:
        _score_matvec(st, ci - 1)
    if ci == NCH - 1:
        _score_matvec(st, ci)


def _deinterleave(st, row_lo, row_hi):
    st.mark(f'deint_{row_lo}_{row_hi}')
    nc = st.nc
    if st.scores_e is None:
        st.scores_e = st.singles.tile([128, R], f32)
        st.scores_o = st.singles.tile([128, R], f32)
    n = row_hi - row_lo
    src = st.scores_tm
    se = bass.AP(tensor=src.tensor, offset=src.offset + 2 * row_lo,
                 ap=[list(src.ap[0]), [2, n]])
    so = bass.AP(tensor=src.tensor, offset=src.offset + 2 * row_lo + 1,
                 ap=[list(src.ap[0]), [2, n]])
    nc.vector.tensor_copy(st.scores_e[:, row_lo:row_hi], se)
    nc.vector.tensor_copy(st.scores_o[:, row_lo:row_hi], so)


def _prefix(st, scope):
    """Masked row-major scores for a scope: transpose + mask fill.
    The scope's mask/fill tiles convert just-in-time on Pool."""
    st.mark(f'prefix_{scope}')
    nc = st.nc
    lo, hi, _ = SCOPES[scope]
    rg = hi - lo
    mfs = st.singles.tile([rg, T], f32, tag=f"mf{scope}", name=f"mf{scope}")
    nc.gpsimd.tensor_copy(mfs, st.mask_u[scope])
    ft = st.singles.tile([rg, T], f32, tag=f"ft{scope}", name=f"ft{scope}")
    nc.gpsimd.tensor_scalar(
        out=ft, in0=mfs, scalar1=1.0, scalar2=BIG,
        op0=Alu.subtract, op1=Alu.mult,
    )
    st.mask_f[scope] = mfs
    st.fillt[scope] = ft
    srm = st.singles.tile([rg, T], f32, tag=f"srm{scope}", name=f"srm{scope}")
    pse = st.psum.tile([rg, 128], f32, tag="psc", name=f"pse{scope}", bufs=2)
    nc.tensor.transpose(pse, st.scores_e[:, lo:hi], st.ident)
    nc.vector.tensor_copy(srm[:, 0:128], pse)
    pso = st.psum.tile([rg, 128], f32, tag="psc", name=f"pso{scope}", bufs=2)
    nc.tensor.transpose(pso, st.scores_o[:, lo:hi], st.ident)
    nc.vector.tensor_copy(srm[:, 128:256], pso)
    msk = st.singles.tile([rg, T], f32, tag=f"smsk{scope}",
                          name=f"smsk{scope}")
    nc.vector.scalar_tensor_tensor(
        out=msk, in0=srm, scalar=1.0, in1=st.mask_f[scope],
        op0=Alu.mult, op1=Alu.mult,
    )
    nc.vector.tensor_add(msk, msk, st.fillt[scope])
    st.srm[scope] = msk
    st.work[scope] = st.singles.tile([rg, T], f32, tag=f"work{scope}",
                                     name=f"work{scope}")
    st.rounds_done[scope] = 0


def _rounds(st, scope, upto):
    st.mark(f'rounds_{scope}_{upto}')
    nc = st.nc
    lo, hi, n_rounds = SCOPES[scope]
    rg = hi - lo
    while st.rounds_done[scope] < upto:
        r = st.rounds_done[scope]
        mx = st.singles.tile([rg, 8], f32, tag=f"mx{scope}_{r}",
                             name=f"mx{scope}_{r}")
        src = st.srm[scope] if r == 0 else st.work[scope]
        nc.vector.max(out=mx, in_=src)
        if r < n_rounds - 1:
            nc.vector.match_replace(
                out=st.work[scope], in_to_replace=mx, in_values=src,
                imm_value=REPL,
            )
        st.last_mx[scope] = mx
        st.rounds_done[scope] += 1
    return st.last_mx[scope][:, 7:8]


def _make_gcols(st, scope, k, thr):
    """diff -> sigmoid -> transpose into interleaved gate cols [128, 2*rg]."""
    st.mark(f'gcols_{scope}_{k}')
    nc = st.nc
    lo, hi, _ = SCOPES[scope]
    rg = hi - lo
    dif = st.singles.tile([rg, T], f32, tag=f"dif{scope}_{k}",
                          name=f"dif{scope}_{k}")
    nc.vector.tensor_scalar(
        out=dif, in0=st.srm[scope], scalar1=thr, scalar2=CLAMP * EMB_SCALE,
        op0=Alu.subtract, op1=Alu.min,
    )
    nc.scalar.activation(dif, dif, Act.Sigmoid, bias=0.0,
                         scale=st.temp_col[scope])
    g = st.singles.tile([128, 2 * rg], f32, tag=f"g{scope}_{k}",
                        name=f"g{scope}_{k}")
    pme = st.psum.tile([128, rg], f32, tag="pst", name=f"pme{scope}{k}",
                       bufs=2)
    nc.tensor.transpose(pme, dif[:, 0:128], st.ident[:rg, :rg])
    ge = bass.AP(tensor=g.tensor, offset=g.offset, ap=[list(g.ap[0]), [2, rg]])
    nc.vector.tensor_copy(ge, pme)
    pmo = st.psum.tile([128, rg], f32, tag="pst", name=f"pmo{scope}{k}",
                       bufs=2)
    nc.tensor.transpose(pmo, dif[:, 128:256], st.ident[:rg, :rg])
    go = bass.AP(tensor=g.tensor, offset=g.offset + 1,
                 ap=[list(g.ap[0]), [2, rg]])
    nc.vector.tensor_copy(go, pmo)
    st.gcols[(scope, k)] = g


def _gate_unit(st, k, unit, scope):
    st.mark(f'gate_{k}_{unit}')
    nc = st.nc
    lo, hi, _ = SCOPES[scope]
    g = st.gcols[(scope, k)]
    och = st.opool.tile([128, UNIT_TILES * D], bf16, tag="och")
    st.ochs[(k, unit)] = och
    t0 = unit * UNIT_TILES
    pat = UNIT_ENGINES[(k, unit)]
    gi = 0
    while gi < 4:
        eng = pat[gi]
        ngr = 1
        if eng == "D":
            while gi + ngr < 4 and pat[gi + ngr] == eng:
                ngr += 1
        ts = t0 + gi * 8
        gofs = ts - 2 * lo
        if eng == "A":
            for j in range(8):
                t = ts + j
                col = g[:, gofs + j:gofs + j + 1]
                nc.scalar.activation(
                    st.och_ap(och, t - t0, 1), st.tile_ap(t), Act.Copy,
                    bias=0.0, scale=col,
                )
        else:
            n = ngr * 8
            gb = bass.AP(tensor=g.tensor, offset=g.offset + gofs,
                         ap=[list(g.ap[0]), [1, n], [0, D]])
            e = nc.vector if eng == "D" else nc.gpsimd
            e.tensor_tensor(
                out=st.och_ap(och, ts - t0, n),
                in0=st.embbuf[:, ts * D:(ts + n) * D],
                in1=gb, op=Alu.mult,
            )
        gi += ngr


def _store_unit(st, k, unit):
    st.mark(f'store_{k}_{unit}')
    nc = st.nc
    k_i = KS.index(k)
    nc.sync.dma_start(
        out=st.out.ap()[k_i, :,
                        unit * UNIT_TILES * D:(unit + 1) * UNIT_TILES * D],
        in_=st.ochs[(k, unit)],
    )


def _emit_pipeline(st):
    _score_chunk(st, 0)
    _score_chunk(st, 1)
    _score_chunk(st, 2)
    # scope A (rows 0-15 = chunks 0-1)
    _deinterleave(st, 0, 16)
    _prefix(st, "A")
    thrA32 = _rounds(st, "A", 4)
    _make_gcols(st, "A", 32, thrA32)
    _score_chunk(st, 3)
    _gate_unit(st, 32, 0, "A")
    _store_unit(st, 32, 0)
    _score_chunk(st, 4)
    # scope B (rows 16-31 = chunks 2-3)
    _deinterleave(st, 16, 32)
    _prefix(st, "B")
    thrB32 = _rounds(st, "B", 4)
    _make_gcols(st, "B", 32, thrB32)
    _score_chunk(st, 5)
    _gate_unit(st, 32, 1, "B")
    _store_unit(st, 32, 1)
    _score_chunk(st, 6)
    thrA64 = _rounds(st, "A", 8)
    _make_gcols(st, "A", 64, thrA64)
    _gate_unit(st, 64, 0, "A")
    _store_unit(st, 64, 0)
    _score_chunk(st, 7)
    thrB64 = _rounds(st, "B", 8)
    _make_gcols(st, "B", 64, thrB64)
    _gate_unit(st, 64, 1, "B")
    _store_unit(st, 64, 1)
    # full chain over all 64 rows supplies every remaining threshold
    _deinterleave(st, 32, 64)
    _prefix(st, "F")
    thrF32 = _rounds(st, "F", 4)
    _make_gcols(st, "F", 32, thrF32)
    _gate_unit(st, 32, 2, "F")
    _store_unit(st, 32, 2)
    _gate_unit(st, 32, 3, "F")
    _store_unit(st, 32, 3)
    thrF64 = _rounds(st, "F", 8)
    _make_gcols(st, "F", 64, thrF64)
    _gate_unit(st, 64, 2, "F")
    _store_unit(st, 64, 2)
    _gate_unit(st, 64, 3, "F")
    _store_unit(st, 64, 3)
    thrF128 = _rounds(st, "F", 16)
    _make_gcols(st, "F", 128, thrF128)
    for u in range(4):
        _gate_unit(st, 128, u, "F")
        _store_unit(st, 128, u)


PHASES = []


_NC = None


def _get_nc():
    global _NC
    if _NC is None:
        _NC = build_bass()
    return _NC


def make_in_maps(embeddings, w, temperature, mask):
    """Shard + device-layout the full inputs for the 8 cores."""
    emb = np.asarray(embeddings, dtype=np.float32)
    w = np.ascontiguousarray(np.asarray(w, dtype=np.float32))
    temp = np.ascontiguousarray(np.asarray(temperature, dtype=np.float32))
    mask_u8 = np.asarray(mask).astype(np.uint8)
    in_maps = []
    for c in range(N_CORES):
        sl = slice(c * R, (c + 1) * R)
        esh = emb[sl].reshape(NT, 128, D).transpose(1, 0, 2).reshape(128, NT * D)
        if EMB_SCALE != 1.0:
            esh = esh * EMB_SCALE
        in_maps.append({
            "emb_tm": np.ascontiguousarray(esh.astype(EMB_NP)),
            "w": w,
            "temperature": temp,
            "mask": np.ascontiguousarray(mask_u8[sl]),
        })
    return in_maps


def postprocess(results):
    """Device bf16 [3, 128, NT*D] outputs -> full [3, B, T, D] f32."""
    outs = []
    for r in results:
        o = np.asarray(r["out"]).astype(np.float32)
        if EMB_SCALE != 1.0:
            o *= 1.0 / EMB_SCALE
        o = o.reshape(len(KS), 128, NT, D).transpose(0, 2, 1, 3)
        outs.append(o.reshape(len(KS), R, T, D))
    return np.concatenate(outs, axis=1)


def kernel(embeddings, w, b, temperature, mask):
    nc = _get_nc()
    in_maps = make_in_maps(embeddings, w, temperature, mask)
    res = run_bass_kernel_spmd(nc, in_maps, core_ids=list(range(N_CORES)))
    return postprocess(res.results)
